# revision 5
# baseline (speedup 1.0000x reference)
"""Relative multi-head attention Trainium2 kernel (8 NeuronCores, SPMD).

Math: the position MLP has no nonlinearity, so
    a_k = pos_mat @ M + c,  M = rp_w2 @ rp_w1 (64,2), c = rp_w2 @ rp_b1 + rp_b2
and the einsum 'bhqd,bhkd,qkd->bhqk' decomposes into three plain score matmuls
    L = pos0 * S0 + pos1 * S1 + Sb,   Sp = (Q ⊙ M[:,p]) @ K^T,  Sb = (Q ⊙ c) @ K^T
Everything on-chip is computed in a transposed layout (k on partitions, q on
the free dim): scores S^T, softmax sums via a ones-column folded into the PV
matmul, PV -> out^T which feeds the fc matmul directly as the stationary
operand.  Softmax skips the max-subtraction (logits are ~1e-2).

Sharding: 8 cores = 4 batches x 2 query-halves; fully local, no collectives.
"""

import os
import numpy as np

import concourse.bass as bass
import concourse.bacc as bacc
import concourse.mybir as mybir
import concourse.tile as tile
from concourse.bass import ds, ts
from concourse.bass_utils import run_bass_kernel_spmd
from concourse.masks import make_identity

F32 = mybir.dt.float32
F16 = mybir.dt.float16
AF = mybir.ActivationFunctionType
OP = mybir.AluOpType

B, S, DM, H, DK = 4, 1024, 512, 8, 64
TEMP = 8.0
LN_EPS = 1e-6
QH = S // 2          # q rows per core (q-half)
N_CORES = 8

LAST_RESULT = None   # set by kernel() for test harness introspection


def _build():
    nc = bacc.Bacc("TRN2", target_bir_lowering=False, debug=False)

    # ---- DRAM I/O ----
    def din(name, shape, dt=F16):
        return nc.dram_tensor(name, list(shape), dt, kind="ExternalInput").ap()

    qT = din("qT", (DM, QH))             # q[b].T slice         (m, q)
    kT = din("kT", (DM, S))              # k[b].T               (m, k)
    vT = din("vT", (DM, S))              # v[b].T               (m, k)
    wqsT = din("wqsT", (DM, DM))         # w_qs.T               (m, hd)
    wksT = din("wksT", (DM, DM))
    wvsT = din("wvsT", (DM, DM))
    wfcT = din("wfcT", (DM, DM))         # w_fc.T               (hd, dm)
    m0c = din("m0c", (128, 1), F32)      # M[p%64,0] scale column
    m1c = din("m1c", (128, 1), F32)
    mbc = din("mbc", (128, 1), F32)
    pos0T = din("pos0T", (S, QH))        # pos_mat[qsl,:,0].T   (k, q)
    pos1T = din("pos1T", (S, QH))
    resid = din("resid", (QH, DM), F32)  # q[b][qsl]            (q, dm)
    lng = din("lng", (1, DM), F32)
    lnb = din("lnb", (1, DM), F32)

    attnT = nc.dram_tensor("attnT", [H, S, QH], F32, kind="ExternalOutput").ap()
    outp = nc.dram_tensor("outp", [QH, DM], F32, kind="ExternalOutput").ap()

    with tile.TileContext(nc) as tc:
        _body(tc, nc, qT, kT, vT, wqsT, wksT, wvsT, wfcT, m0c, m1c, mbc,
              pos0T, pos1T, resid, lng, lnb, attnT, outp)
    nc.compile()
    return nc


def _body(tc, nc, qT, kT, vT, wqsT, wksT, wvsT, wfcT, m0c, m1c, mbc,
          pos0T, pos1T, resid, lng, lnb, attnT, outp):
    with (
        tc.tile_pool(name="consts", bufs=1) as consts,
        tc.tile_pool(name="dram", bufs=1, space="DRAM") as drampool,
    ):
        # ---- load constants / inputs ----
        def load4(pool, dram_ap, free, dt=F16, tag=None):
            t = pool.tile([128, 4, free], dt, tag=tag)
            nc.sync.dma_start(out=t, in_=dram_ap.rearrange("(c p) f -> p c f", p=128))
            return t

        wfc_s = load4(consts, wfcT, DM, tag="wfc")

        pos0_s = consts.tile([128, 8, QH], F16, tag="pos0")
        nc.sync.dma_start(out=pos0_s, in_=pos0T.rearrange("(c p) q -> p c q", p=128))
        pos1_s = consts.tile([128, 8, QH], F16, tag="pos1")
        nc.sync.dma_start(out=pos1_s, in_=pos1T.rearrange("(c p) q -> p c q", p=128))

        m0_s = consts.tile([128, 1], F32, tag="m0")
        nc.sync.dma_start(out=m0_s, in_=m0c)
        m1_s = consts.tile([128, 1], F32, tag="m1")
        nc.sync.dma_start(out=m1_s, in_=m1c)
        mb_s = consts.tile([128, 1], F32, tag="mb")
        nc.sync.dma_start(out=mb_s, in_=mbc)

        ident = consts.tile([128, 128], F16, tag="ident")
        make_identity(nc, ident)

        resid_s = consts.tile([128, 4, DM], F32, tag="resid")
        nc.sync.dma_start(out=resid_s, in_=resid.rearrange("(c p) d -> p c d", p=128))
        g_rep = consts.tile([128, DM], F32, tag="grep")
        nc.sync.dma_start(out=g_rep, in_=lng.to_broadcast((128, DM)))
        b_rep = consts.tile([128, DM], F32, tag="brep")
        nc.sync.dma_start(out=b_rep, in_=lnb.to_broadcast((128, DM)))
        eps_s = consts.tile([128, 1], F32, tag="eps")
        nc.vector.memset(eps_s, LN_EPS)

        # ---- projections ----
        KT_s = consts.tile([128, 4, S], F16, tag="KT")          # K^T  (hd, k)
        V_s = consts.tile([128, 8, H, DK + 1], F16, tag="V")    # V natural + ones col
        Q0_s = consts.tile([128, 4, QH], F16, tag="Q0")         # Q0^T (hd, q)
        Q1_s = consts.tile([128, 4, QH], F16, tag="Q1")
        Qb_s = consts.tile([128, 4, QH], F16, tag="Qb")

        nc.vector.memset(V_s[:, :, :, DK:DK + 1], 1.0)

        with (
            tc.tile_pool(name="phaseA", bufs=1) as pA,
            tc.tile_pool(name="psA", bufs=3, space="PSUM") as psA,
        ):
            wqs_s = load4(pA, wqsT, DM, tag="wqs")
            wks_s = load4(pA, wksT, DM, tag="wks")
            wvs_s = load4(pA, wvsT, DM, tag="wvs")
            kT_s = load4(pA, kT, S, tag="kTs")
            vT_s = load4(pA, vT, S, tag="vTs")
            qT_s = load4(pA, qT, QH, tag="qTs")
            for hdch in range(4):
                for half in range(2):
                    ps = psA.tile([128, 512], F32, tag="proj")
                    for mch in range(4):
                        nc.tensor.matmul(
                            ps,
                            wks_s[:, mch, ts(hdch, 128)],
                            kT_s[:, mch, ds(half * 512, 512)],
                            start=(mch == 0), stop=(mch == 3),
                        )
                    nc.scalar.activation(
                        out=KT_s[:, hdch, ds(half * 512, 512)], in_=ps, func=AF.Copy)
            for kch in range(8):
                ps = psA.tile([128, 512], F32, tag="proj")
                for mch in range(4):
                    nc.tensor.matmul(
                        ps,
                        vT_s[:, mch, ts(kch, 128)],
                        wvs_s[:, mch, :],
                        start=(mch == 0), stop=(mch == 3),
                    )
                nc.scalar.activation(
                    out=V_s[:, kch, :, 0:DK],
                    in_=ps.rearrange("p (h d) -> p h d", h=H), func=AF.Copy)
            for hdch in range(4):
                ps = psA.tile([128, 512], F32, tag="proj")
                for mch in range(4):
                    nc.tensor.matmul(
                        ps,
                        wqs_s[:, mch, ts(hdch, 128)],
                        qT_s[:, mch, :],
                        start=(mch == 0), stop=(mch == 3),
                    )
                nc.scalar.activation(out=Q0_s[:, hdch, :], in_=ps, func=AF.Copy, scale=m0_s)
                nc.scalar.activation(out=Q1_s[:, hdch, :], in_=ps, func=AF.Copy, scale=m1_s)
                nc.scalar.activation(out=Qb_s[:, hdch, :], in_=ps, func=AF.Copy, scale=mb_s)

        # ---- attention ----
        outTu_s = consts.tile([128, 4, QH], F16, tag="outTu")   # unnormalized out^T
        sums_s = consts.tile([H, QH], F32, tag="sums")
        P_tiles = {}
        with (
            tc.tile_pool(name="psS", bufs=2, space="PSUM") as psS,
            tc.tile_pool(name="psL", bufs=2, space="PSUM") as psL,
            tc.tile_pool(name="psPV", bufs=2, space="PSUM") as psPV,
            tc.tile_pool(name="tu", bufs=4) as tu_pool,
            tc.tile_pool(name="sums", bufs=2) as sums_pool,
            tc.tile_pool(name="pall", bufs=64) as pall,
        ):
            for h in range(H):
                hr = (h % 2) * 64
                hc = h // 2
                pv = psPV.tile([DK + 1, QH], F32, tag="pv")
                for kch in range(8):
                    klhs = KT_s[ds(hr, 64), hc, ts(kch, 128)]
                    s0 = psS.tile([128, QH], F32, tag="s0")
                    s1 = psS.tile([128, QH], F32, tag="s1")
                    lps = psL.tile([128, QH], F32, tag="lps")
                    nc.tensor.matmul(s0, klhs, Q0_s[ds(hr, 64), hc, :])
                    nc.tensor.matmul(s1, klhs, Q1_s[ds(hr, 64), hc, :])
                    nc.tensor.matmul(lps, klhs, Qb_s[ds(hr, 64), hc, :],
                                     start=True, stop=False, skip_group_check=True)
                    t_s = tu_pool.tile([128, QH], F16, tag="t")
                    u_s = tu_pool.tile([128, QH], F16, tag="u")
                    nc.vector.tensor_tensor(out=t_s, in0=s0, in1=pos0_s[:, kch, :], op=OP.mult)
                    nc.vector.tensor_tensor(out=u_s, in0=s1, in1=pos1_s[:, kch, :], op=OP.mult)
                    nc.tensor.matmul(lps, ident, t_s,
                                     start=False, stop=False, skip_group_check=True)
                    nc.tensor.matmul(lps, ident, u_s,
                                     start=False, stop=True, skip_group_check=True)
                    pt = pall.tile([128, QH], F16, tag="P")
                    P_tiles[(h, kch)] = pt
                    nc.scalar.activation(out=pt, in_=lps, func=AF.Exp, scale=1.0 / TEMP)
                    nc.tensor.matmul(pv, V_s[:, kch, h, :], pt,
                                     start=(kch == 0), stop=(kch == 7),
                                     skip_group_check=True)
                nc.scalar.activation(out=outTu_s[ds(hr, 64), hc, :],
                                     in_=pv[0:DK, :], func=AF.Copy)
                st = sums_pool.tile([128, QH], F32, tag="sumstage")
                nc.scalar.activation(out=st[ds(DK, 1), :], in_=pv[ds(DK, 1), :],
                                     func=AF.Copy)
                nc.sync.dma_start(out=sums_s[ds(h, 1), :], in_=st[ds(DK, 1), :])

            # ---- normalization factors ----
            recip_s = consts.tile([H, QH], F32, tag="recip")
            nc.vector.reciprocal(recip_s, sums_s)
            recip16 = consts.tile([H, QH], F16, tag="recip16")
            nc.vector.tensor_copy(out=recip16, in_=recip_s)
            recip_dram = drampool.tile([H, QH], F16, tag="recipd")
            nc.sync.dma_start(out=recip_dram, in_=recip16)
            rep_s = consts.tile([128, H, QH], F16, tag="rep")
            rep2_s = consts.tile([128, 4, QH], F16, tag="rep2")
            for h in range(H):
                nc.gpsimd.dma_start(
                    out=rep_s[:, h, :],
                    in_=recip_dram[ds(h, 1), :].to_broadcast((128, QH)))
                nc.gpsimd.dma_start(
                    out=rep2_s[ds((h % 2) * 64, 64), h // 2, :],
                    in_=recip_dram[ds(h, 1), :].to_broadcast((64, QH)))

            # ---- attn output: normalize + store ----
            with tc.tile_pool(name="astage", bufs=6) as astage:
                for h in range(H):
                    for kch in range(8):
                        a_s = astage.tile([128, QH], F32, tag="a")
                        eng = nc.gpsimd if (h * 8 + kch) % 2 == 0 else nc.vector
                        eng.tensor_tensor(out=a_s, in0=P_tiles[(h, kch)],
                                          in1=rep_s[:, h, :], op=OP.mult)
                        nc.sync.dma_start(out=attnT[h, ts(kch, 128), :], in_=a_s)

        # ---- fc + residual + layernorm ----
        outT_s = consts.tile([128, 4, QH], F16, tag="outT")
        for hdch in range(4):
            nc.vector.tensor_tensor(out=outT_s[:, hdch, :], in0=outTu_s[:, hdch, :],
                                    in1=rep2_s[:, hdch, :], op=OP.mult)
        with (
            tc.tile_pool(name="psFC", bufs=2, space="PSUM") as psFC,
            tc.tile_pool(name="ln", bufs=3) as ln_pool,
        ):
            for qch in range(4):
                fc = psFC.tile([128, DM], F32, tag="fc")
                for hdch in range(4):
                    nc.tensor.matmul(fc, outT_s[:, hdch, ts(qch, 128)],
                                     wfc_s[:, hdch, :],
                                     start=(hdch == 0), stop=(hdch == 3))
                x_s = ln_pool.tile([128, DM], F32, tag="x")
                nc.vector.tensor_tensor(out=x_s, in0=fc, in1=resid_s[:, qch, :], op=OP.add)
                stats = ln_pool.tile([128, 6], F32, tag="stats")
                nc.vector.bn_stats(out=stats, in_=x_s)
                mv = ln_pool.tile([128, 2], F32, tag="mv")
                nc.vector.bn_aggr(out=mv, in_=stats)
                std_s = ln_pool.tile([128, 1], F32, tag="std")
                nc.scalar.activation(out=std_s, in_=mv[:, 1:2], func=AF.Sqrt, bias=eps_s)
                rstd_s = ln_pool.tile([128, 1], F32, tag="rstd")
                nc.vector.reciprocal(rstd_s, std_s)
                y_s = ln_pool.tile([128, DM], F32, tag="y")
                nc.vector.tensor_scalar(out=y_s, in0=x_s, scalar1=mv[:, 0:1],
                                        scalar2=rstd_s, op0=OP.subtract, op1=OP.mult)
                y2_s = ln_pool.tile([128, DM], F32, tag="y2")
                nc.vector.scalar_tensor_tensor(out=y2_s, in0=y_s, scalar=1.0,
                                               in1=g_rep, op0=OP.bypass, op1=OP.mult)
                y3_s = ln_pool.tile([128, DM], F32, tag="y3")
                nc.vector.tensor_tensor(out=y3_s, in0=y2_s, in1=b_rep, op=OP.add)
                nc.sync.dma_start(out=outp[ts(qch, 128), :], in_=y3_s)


_NC_CACHE = None


def _get_nc():
    global _NC_CACHE
    if _NC_CACHE is None:
        _NC_CACHE = _build()
    return _NC_CACHE


def kernel(q, k, v, pos_mat, w_qs, w_ks, w_vs, w_fc,
           rp_w1, rp_b1, rp_w2, rp_b2, ln_g, ln_b):
    global LAST_RESULT
    f32 = np.float32
    f16 = np.float16
    q = np.asarray(q, f32)
    k = np.asarray(k, f32)
    v = np.asarray(v, f32)
    pos_mat = np.asarray(pos_mat, f32)
    w_qs = np.asarray(w_qs, f32)
    w_ks = np.asarray(w_ks, f32)
    w_vs = np.asarray(w_vs, f32)
    w_fc = np.asarray(w_fc, f32)
    rp_w1 = np.asarray(rp_w1, f32)
    rp_b1 = np.asarray(rp_b1, f32)
    rp_w2 = np.asarray(rp_w2, f32)
    rp_b2 = np.asarray(rp_b2, f32)
    ln_g = np.asarray(ln_g, f32)
    ln_b = np.asarray(ln_b, f32)

    # fold the (linear) position MLP:  a_k = pos_mat @ M.T-ish + c
    M = rp_w2 @ rp_w1                      # (64, 2)
    cvec = rp_w2 @ rp_b1 + rp_b2           # (64,)
    p64 = np.arange(128) % 64
    m0c = np.ascontiguousarray(M[p64, 0].reshape(128, 1), dtype=f32)
    m1c = np.ascontiguousarray(M[p64, 1].reshape(128, 1), dtype=f32)
    mbc = np.ascontiguousarray(cvec[p64].reshape(128, 1), dtype=f32)

    shared = {
        "wqsT": np.ascontiguousarray(w_qs.T, dtype=f16),
        "wksT": np.ascontiguousarray(w_ks.T, dtype=f16),
        "wvsT": np.ascontiguousarray(w_vs.T, dtype=f16),
        "wfcT": np.ascontiguousarray(w_fc.T, dtype=f16),
        "m0c": m0c, "m1c": m1c, "mbc": mbc,
        "lng": np.ascontiguousarray(ln_g.reshape(1, DM), dtype=f32),
        "lnb": np.ascontiguousarray(ln_b.reshape(1, DM), dtype=f32),
    }
    kT_b = [np.ascontiguousarray(k[b].T, dtype=f16) for b in range(B)]
    vT_b = [np.ascontiguousarray(v[b].T, dtype=f16) for b in range(B)]
    qT_b = [np.ascontiguousarray(q[b].T, dtype=f16) for b in range(B)]

    in_maps = []
    for c in range(N_CORES):
        b = c % B
        qh = c // B
        qsl = slice(qh * QH, (qh + 1) * QH)
        in_maps.append(dict(
            shared,
            qT=np.ascontiguousarray(qT_b[b][:, qsl]),
            kT=kT_b[b],
            vT=vT_b[b],
            pos0T=np.ascontiguousarray(pos_mat[qsl, :, 0].T, dtype=f16),
            pos1T=np.ascontiguousarray(pos_mat[qsl, :, 1].T, dtype=f16),
            resid=np.ascontiguousarray(q[b, qsl], dtype=f32),
        ))

    nc = _get_nc()
    res = run_bass_kernel_spmd(nc, in_maps, core_ids=list(range(N_CORES)))
    LAST_RESULT = res

    out = np.empty((B, S, DM), f32)
    attn = np.empty((B, H, S, S), f32)
    for c in range(N_CORES):
        b = c % B
        qh = c // B
        qsl = slice(qh * QH, (qh + 1) * QH)
        r = res.results[c]
        out[b, qsl] = r["outp"]
        attn[b, :, qsl, :] = r["attnT"].transpose(0, 2, 1)
    return out, attn
